# revision 1
# baseline (speedup 1.0000x reference)
"""GumbelSelector Trainium2 kernel.

Math: h = relu(s @ W1 + b1); lo = h @ W2 + b2  (2 classes)
  dec  = (argmax(lo) == 1)  ==  (z > 0)         where z = h @ (W2[:,1]-W2[:,0]) + (b2[1]-b2[0])
  prob = softmax(lo)[..., 1] ==  sigmoid(z)
  Per-row correction (LB=1): if a row of dec is all zero, activate argmax(rnoise).

Sharding: data-parallel over batch B=64 -> 8 cores x 8 rows. Weights replicated.
Host pre-transposes each core's s shard to [D=256, 32768] so the DMA loads are
fully coalesced and the contraction dim lands on SBUF partitions directly.
"""

import sys

if "/opt/trn_rl_repo" not in sys.path:
    sys.path.insert(0, "/opt/trn_rl_repo")

import numpy as np

import concourse.bass as bass
import concourse.mybir as mybir
import concourse.tile as tile
from concourse import bacc
from concourse.bass_utils import run_bass_kernel_spmd

B, N, D = 64, 4096, 256
HID = D // 2  # 128
NCORES = 8
BPC = B // NCORES          # batch rows per core
TOK = BPC * N              # 32768 tokens per core
SLAB = 2048                # tokens per DMA slab (1 MiB per 128-partition load)
TS = 1024                  # tokens per compute tile (2 PSUM banks)
F32 = mybir.dt.float32

_NC = None


def _build_nc():
    nc = bacc.Bacc("TRN2", target_bir_lowering=False, debug=False)
    sT = nc.dram_tensor("sT", [D, TOK], F32, kind="ExternalInput")
    rn = nc.dram_tensor("rn", [BPC, N], F32, kind="ExternalInput")
    w1 = nc.dram_tensor("w1", [D, HID], F32, kind="ExternalInput")
    b1 = nc.dram_tensor("b1", [HID, 1], F32, kind="ExternalInput")
    w2d = nc.dram_tensor("w2d", [HID, 1], F32, kind="ExternalInput")
    b2d = nc.dram_tensor("b2d", [1, 1], F32, kind="ExternalInput")
    nb2d = nc.dram_tensor("nb2d", [1, 1], F32, kind="ExternalInput")
    dec = nc.dram_tensor("dec", [1, TOK], F32, kind="ExternalOutput")
    prob = nc.dram_tensor("prob", [1, TOK], F32, kind="ExternalOutput")

    AF = mybir.ActivationFunctionType
    ALU = mybir.AluOpType

    with tile.TileContext(nc) as tc:
        with (
            tc.tile_pool(name="consts", bufs=1) as consts,
            tc.tile_pool(name="io8", bufs=1) as io8,
            tc.tile_pool(name="sload", bufs=3) as sload,
            tc.tile_pool(name="hpool", bufs=3) as hpool,
            tc.tile_pool(name="cpool", bufs=4) as cpool,
            tc.tile_pool(name="phpool", bufs=2, space=bass.MemorySpace.PSUM) as phpool,
            tc.tile_pool(name="pzpool", bufs=2, space=bass.MemorySpace.PSUM) as pzpool,
        ):
            w1a = consts.tile([128, HID], F32)
            nc.sync.dma_start(w1a[:], w1[0:128, :])
            w1b = consts.tile([128, HID], F32)
            nc.sync.dma_start(w1b[:], w1[128:256, :])
            b1s = consts.tile([HID, 1], F32)
            nc.sync.dma_start(b1s[:], b1[:])
            w2s = consts.tile([HID, 1], F32)
            nc.sync.dma_start(w2s[:], w2d[:])
            b2s = consts.tile([1, 1], F32)
            nc.sync.dma_start(b2s[:], b2d[:])
            nb2s = consts.tile([1, 1], F32)
            nc.sync.dma_start(nb2s[:], nb2d[:])
            rns = io8.tile([BPC, N], F32)
            nc.sync.dma_start(rns[:], rn[:])

            # engines may only address base partition 0/32/64/96, so compute
            # dec/prob chunks on partition 0; prob streams straight to DRAM,
            # dec chunks are DMA'd into row-layout for the row fixup
            dec8 = io8.tile([BPC, N], F32)

            for si in range(TOK // SLAB):
                off = si * SLAB
                sa = sload.tile([128, SLAB], F32, tag="sa")
                sb = sload.tile([128, SLAB], F32, tag="sb")
                nc.sync.dma_start(sa[:], sT[0:128, off : off + SLAB])
                nc.sync.dma_start(sb[:], sT[128:256, off : off + SLAB])
                for half in range(SLAB // TS):
                    toff = off + half * TS
                    hoff = half * TS
                    ph = phpool.tile([128, TS], F32)
                    # same stationary back to back to minimize LDWEIGHTS swaps
                    nc.tensor.matmul(ph[:, 0:512], w1a[:], sa[:, hoff : hoff + 512],
                                     start=True, stop=False)
                    nc.tensor.matmul(ph[:, 512:1024], w1a[:], sa[:, hoff + 512 : hoff + 1024],
                                     start=True, stop=False)
                    nc.tensor.matmul(ph[:, 0:512], w1b[:], sb[:, hoff : hoff + 512],
                                     start=False, stop=True)
                    nc.tensor.matmul(ph[:, 512:1024], w1b[:], sb[:, hoff + 512 : hoff + 1024],
                                     start=False, stop=True)
                    h = hpool.tile([128, TS], F32)
                    nc.scalar.activation(h[:], ph[:], AF.Relu, bias=b1s[:])
                    pz = pzpool.tile([1, TS], F32)
                    nc.tensor.matmul(pz[0:1, 0:512], w2s[:], h[:, 0:512],
                                     start=True, stop=True)
                    nc.tensor.matmul(pz[0:1, 512:1024], w2s[:], h[:, 512:1024],
                                     start=True, stop=True)
                    pc = cpool.tile([1, TS], F32, tag="pc")
                    nc.scalar.activation(pc[:], pz[0:1, :], AF.Sigmoid, bias=b2s[:])
                    nc.sync.dma_start(prob[0:1, toff : toff + TS], pc[:])
                    dc = cpool.tile([1, TS], F32, tag="dc")
                    nc.vector.tensor_scalar(dc[:], pz[0:1, :], nb2s[:], None, ALU.is_gt)
                    b_row, col = toff // N, toff % N
                    nc.sync.dma_start(dec8[b_row : b_row + 1, col : col + TS], dc[:])

            # Row correction: rows with no active slot get argmax(rnoise) forced on.
            rmaxd = io8.tile([BPC, 1], F32)
            nc.vector.tensor_reduce(rmaxd[:], dec8[:], mybir.AxisListType.X, ALU.max)
            need = io8.tile([BPC, 1], F32)
            nc.vector.tensor_scalar(need[:], rmaxd[:], 0.0, None, ALU.is_equal)
            rmaxr = io8.tile([BPC, 1], F32)
            nc.vector.tensor_reduce(rmaxr[:], rns[:], mybir.AxisListType.X, ALU.max)
            fix = io8.tile([BPC, N], F32)
            nc.vector.tensor_scalar(fix[:], rns[:], rmaxr[:], need[:],
                                    ALU.is_equal, ALU.mult)
            decf = io8.tile([BPC, N], F32)
            nc.vector.tensor_max(decf[:], dec8[:], fix[:])

            for b in range(BPC):
                nc.sync.dma_start(dec[0:1, b * N : (b + 1) * N], decf[b : b + 1, :])

    nc.compile()
    return nc


def _get_nc():
    global _NC
    if _NC is None:
        _NC = _build_nc()
    return _NC


def _make_in_maps(s, W1, b1, W2, b2, rnoise):
    s = np.ascontiguousarray(s, dtype=np.float32)
    w1 = np.ascontiguousarray(W1, dtype=np.float32)
    b1c = np.ascontiguousarray(b1, dtype=np.float32).reshape(HID, 1)
    w2dc = np.ascontiguousarray(W2[:, 1] - W2[:, 0], dtype=np.float32).reshape(HID, 1)
    b2dv = np.float32(b2[1] - b2[0])
    b2dc = np.array([[b2dv]], dtype=np.float32)
    nb2dc = np.array([[-b2dv]], dtype=np.float32)
    rn = np.ascontiguousarray(rnoise, dtype=np.float32)

    # [NCORES, D, TOK] with the contraction dim outer -> coalesced loads
    sT = np.ascontiguousarray(
        s.reshape(NCORES, TOK, D).transpose(0, 2, 1)
    )
    return [
        {
            "sT": sT[c],
            "rn": rn.reshape(NCORES, BPC, N)[c],
            "w1": w1,
            "b1": b1c,
            "w2d": w2dc,
            "b2d": b2dc,
            "nb2d": nb2dc,
        }
        for c in range(NCORES)
    ]


def run(s, W1, b1, W2, b2, rnoise, trace=False):
    nc = _get_nc()
    in_maps = _make_in_maps(s, W1, b1, W2, b2, rnoise)
    res = run_bass_kernel_spmd(nc, in_maps, list(range(NCORES)), trace=trace)
    dec = np.concatenate(
        [r["dec"].reshape(BPC, N) for r in res.results], axis=0
    )
    prob = np.concatenate(
        [r["prob"].reshape(BPC, N) for r in res.results], axis=0
    )
    return (dec, prob), res


def kernel(s, W1, b1, W2, b2, rnoise):
    (dec, prob), _ = run(s, W1, b1, W2, b2, rnoise)
    return dec, prob



# revision 6
# speedup vs baseline: 1.1353x; 1.1353x over previous
"""GumbelSelector Trainium2 kernel.

Math: h = relu(s @ W1 + b1); lo = h @ W2 + b2  (2 classes)
  dec  = (argmax(lo) == 1)  ==  (z > 0)         where z = h @ (W2[:,1]-W2[:,0]) + (b2[1]-b2[0])
  prob = softmax(lo)[..., 1] ==  sigmoid(z)
  Per-row correction (LB=1): if a row of dec is all zero, activate argmax(rnoise).

Sharding: data-parallel over batch B=64 -> 8 cores x 8 rows. Weights replicated.
Host pre-transposes each core's s shard to [D=256, 32768] so the DMA loads are
fully coalesced and the contraction dim lands on SBUF partitions directly.
"""

import sys

if "/opt/trn_rl_repo" not in sys.path:
    sys.path.insert(0, "/opt/trn_rl_repo")

import numpy as np

import concourse.bass as bass
import concourse.mybir as mybir
import concourse.tile as tile
from concourse import bacc
from concourse.bass_utils import run_bass_kernel_spmd

B, N, D = 64, 4096, 256
HID = D // 2  # 128
NCORES = 8
BPC = B // NCORES          # batch rows per core
TOK = BPC * N              # 32768 tokens per core
SLAB = 2048                # tokens per DMA slab (1 MiB per 128-partition load)
TS = 1024                  # tokens per compute tile (2 PSUM banks)
F32 = mybir.dt.float32
F32R = mybir.dt.float32r   # 1 cycle/row on the PE (vs 4 for fp32) at free>=256

_NC = None


def _build_nc():
    nc = bacc.Bacc("TRN2", target_bir_lowering=False, debug=False)
    sT = nc.dram_tensor("sT", [D, TOK], F32R, kind="ExternalInput")
    rn = nc.dram_tensor("rn", [BPC, N], F32, kind="ExternalInput")
    w1 = nc.dram_tensor("w1", [D, HID], F32R, kind="ExternalInput")
    b1 = nc.dram_tensor("b1", [HID, 1], F32, kind="ExternalInput")
    w2d = nc.dram_tensor("w2d", [HID, 1], F32R, kind="ExternalInput")
    b2d = nc.dram_tensor("b2d", [1, 1], F32, kind="ExternalInput")
    nb2d = nc.dram_tensor("nb2d", [1, 1], F32, kind="ExternalInput")
    dec = nc.dram_tensor("dec", [1, TOK], F32, kind="ExternalOutput")
    prob = nc.dram_tensor("prob", [1, TOK], F32, kind="ExternalOutput")

    AF = mybir.ActivationFunctionType
    ALU = mybir.AluOpType

    with tile.TileContext(nc) as tc:
        with (
            tc.tile_pool(name="consts", bufs=1) as consts,
            tc.tile_pool(name="io8", bufs=1) as io8,
            tc.tile_pool(name="sload", bufs=3) as sload,
            tc.tile_pool(name="hpool", bufs=3) as hpool,
            tc.tile_pool(name="cpool", bufs=4) as cpool,
            tc.tile_pool(name="phpool", bufs=2, space=bass.MemorySpace.PSUM) as phpool,
            tc.tile_pool(name="pzpool", bufs=2, space=bass.MemorySpace.PSUM) as pzpool,
        ):
            w1a = consts.tile([128, HID], F32R)
            nc.sync.dma_start(w1a[:], w1[0:128, :])
            w1b = consts.tile([128, HID], F32R)
            nc.sync.dma_start(w1b[:], w1[128:256, :])
            b1s = consts.tile([HID, 1], F32)
            nc.sync.dma_start(b1s[:], b1[:])
            w2s = consts.tile([HID, 1], F32R)
            nc.sync.dma_start(w2s[:], w2d[:])
            b2s = consts.tile([1, 1], F32)
            nc.sync.dma_start(b2s[:], b2d[:])
            nb2s = consts.tile([1, 1], F32)
            nc.sync.dma_start(nb2s[:], nb2d[:])
            rns = io8.tile([BPC, N], F32)
            nc.sync.dma_start(rns[:], rn[:])

            # engines may only address base partition 0/32/64/96, so compute
            # dec/prob chunks on partition 0; prob streams straight to DRAM,
            # dec chunks are DMA'd into row-layout for the row fixup
            dec8 = io8.tile([BPC, N], F32)

            for si in range(TOK // SLAB):
                off = si * SLAB
                sa = sload.tile([128, SLAB], F32R, tag="sa")
                sb = sload.tile([128, SLAB], F32R, tag="sb")
                nc.sync.dma_start(sa[:], sT[0:128, off : off + SLAB])
                nc.sync.dma_start(sb[:], sT[128:256, off : off + SLAB])
                for half in range(SLAB // TS):
                    toff = off + half * TS
                    hoff = half * TS
                    ph = phpool.tile([128, TS], F32)
                    # same stationary back to back to minimize LDWEIGHTS swaps
                    nc.tensor.matmul(ph[:, 0:512], w1a[:],
                                     sa[:, hoff : hoff + 512],
                                     start=True, stop=False)
                    nc.tensor.matmul(ph[:, 512:1024], w1a[:],
                                     sa[:, hoff + 512 : hoff + 1024],
                                     start=True, stop=False)
                    nc.tensor.matmul(ph[:, 0:512], w1b[:],
                                     sb[:, hoff : hoff + 512],
                                     start=False, stop=True)
                    nc.tensor.matmul(ph[:, 512:1024], w1b[:],
                                     sb[:, hoff + 512 : hoff + 1024],
                                     start=False, stop=True)
                    h = hpool.tile([128, TS], F32R)
                    nc.scalar.activation(h[:], ph[:], AF.Relu, bias=b1s[:])
                    pz = pzpool.tile([1, TS], F32)
                    nc.tensor.matmul(pz[0:1, 0:512], w2s[:],
                                     h[:, 0:512],
                                     start=True, stop=True)
                    nc.tensor.matmul(pz[0:1, 512:1024], w2s[:],
                                     h[:, 512:1024],
                                     start=True, stop=True)
                    pc = cpool.tile([1, TS], F32, tag="pc")
                    nc.scalar.activation(pc[:], pz[0:1, :], AF.Sigmoid, bias=b2s[:])
                    nc.sync.dma_start(prob[0:1, toff : toff + TS], pc[:])
                    dc = cpool.tile([1, TS], F32, tag="dc")
                    nc.vector.tensor_scalar(dc[:], pz[0:1, :], nb2s[:], None, ALU.is_gt)
                    b_row, col = toff // N, toff % N
                    nc.sync.dma_start(dec8[b_row : b_row + 1, col : col + TS], dc[:])

            # Row correction: rows with no active slot get argmax(rnoise) forced on.
            rmaxd = io8.tile([BPC, 1], F32)
            nc.vector.tensor_reduce(rmaxd[:], dec8[:], mybir.AxisListType.X, ALU.max)
            need = io8.tile([BPC, 1], F32)
            nc.vector.tensor_scalar(need[:], rmaxd[:], 0.0, None, ALU.is_equal)
            rmaxr = io8.tile([BPC, 1], F32)
            nc.vector.tensor_reduce(rmaxr[:], rns[:], mybir.AxisListType.X, ALU.max)
            fix = io8.tile([BPC, N], F32)
            nc.vector.tensor_scalar(fix[:], rns[:], rmaxr[:], need[:],
                                    ALU.is_equal, ALU.mult)
            decf = io8.tile([BPC, N], F32)
            nc.vector.tensor_max(decf[:], dec8[:], fix[:])

            for b in range(BPC):
                nc.sync.dma_start(dec[0:1, b * N : (b + 1) * N], decf[b : b + 1, :])

    nc.compile()
    return nc


def _get_nc():
    global _NC
    if _NC is None:
        _NC = _build_nc()
    return _NC


def _make_in_maps(s, W1, b1, W2, b2, rnoise):
    s = np.ascontiguousarray(s, dtype=np.float32)
    w1 = np.ascontiguousarray(W1, dtype=np.float32)
    b1c = np.ascontiguousarray(b1, dtype=np.float32).reshape(HID, 1)
    w2dc = np.ascontiguousarray(W2[:, 1] - W2[:, 0], dtype=np.float32).reshape(HID, 1)
    b2dv = np.float32(b2[1] - b2[0])
    b2dc = np.array([[b2dv]], dtype=np.float32)
    nb2dc = np.array([[-b2dv]], dtype=np.float32)
    rn = np.ascontiguousarray(rnoise, dtype=np.float32)

    # [NCORES, D, TOK] with the contraction dim outer -> coalesced loads
    sT = np.ascontiguousarray(
        s.reshape(NCORES, TOK, D).transpose(0, 2, 1)
    )
    return [
        {
            "sT": sT[c],
            "rn": rn.reshape(NCORES, BPC, N)[c],
            "w1": w1,
            "b1": b1c,
            "w2d": w2dc,
            "b2d": b2dc,
            "nb2d": nb2dc,
        }
        for c in range(NCORES)
    ]


def run(s, W1, b1, W2, b2, rnoise, trace=False):
    nc = _get_nc()
    in_maps = _make_in_maps(s, W1, b1, W2, b2, rnoise)
    res = run_bass_kernel_spmd(nc, in_maps, list(range(NCORES)), trace=trace)
    dec = np.concatenate(
        [r["dec"].reshape(BPC, N) for r in res.results], axis=0
    )
    prob = np.concatenate(
        [r["prob"].reshape(BPC, N) for r in res.results], axis=0
    )
    return (dec, prob), res


def kernel(s, W1, b1, W2, b2, rnoise):
    (dec, prob), _ = run(s, W1, b1, W2, b2, rnoise)
    return dec, prob



# revision 8
# speedup vs baseline: 1.6421x; 1.4464x over previous
"""GumbelSelector Trainium2 kernel.

Math: h = relu(s @ W1 + b1); lo = h @ W2 + b2  (2 classes)
  dec  = (argmax(lo) == 1)  ==  (z > 0)         where z = h @ (W2[:,1]-W2[:,0]) + (b2[1]-b2[0])
  prob = softmax(lo)[..., 1] ==  sigmoid(z)
  Per-row correction (LB=1): if a row of dec is all zero, activate argmax(rnoise).

Sharding: data-parallel over batch B=64 -> 8 cores x 8 rows. Weights replicated.
Host pre-transposes each core's s shard to [D=256, 32768] so the DMA loads are
fully coalesced and the contraction dim lands on SBUF partitions directly.

Main loop is software-pipelined one tile deep: tile i runs layer-1 matmuls
(PE) + relu (ACT) while tile i-1 runs layer-2 matmuls (PE) + sigmoid (ACT).
Matmuls use float32r (1 PE cycle/row at free>=256, vs 4 for fp32).
dec is derived from prob (sigmoid(z) > 0.5  <=>  z > 0) in one bulk DVE op
at the end, so DVE and all small per-tile stores are out of the loop.
"""

import sys

if "/opt/trn_rl_repo" not in sys.path:
    sys.path.insert(0, "/opt/trn_rl_repo")

import numpy as np

import concourse.bass as bass
import concourse.mybir as mybir
import concourse.tile as tile
from concourse import bacc
from concourse.bass_utils import run_bass_kernel_spmd

B, N, D = 64, 4096, 256
HID = D // 2  # 128
NCORES = 8
BPC = B // NCORES          # batch rows per core
TOK = BPC * N              # 32768 tokens per core
SLAB = 2048                # tokens per DMA slab (1 MiB per 128-partition load)
TS = 1024                  # tokens per compute tile (2 PSUM banks)
NT = TOK // TS             # 32 compute tiles
CHUNK = 4096               # prob tokens accumulated per SBUF->SBUF flush
F32 = mybir.dt.float32
F32R = mybir.dt.float32r   # 1 cycle/row on the PE (vs 4 for fp32) at free>=256

_NC = None


def _build_nc():
    nc = bacc.Bacc("TRN2", target_bir_lowering=False, debug=False)
    sT = nc.dram_tensor("sT", [D, TOK], F32R, kind="ExternalInput")
    rn = nc.dram_tensor("rn", [BPC, N], F32, kind="ExternalInput")
    w1 = nc.dram_tensor("w1", [D, HID], F32R, kind="ExternalInput")
    b1 = nc.dram_tensor("b1", [HID, 1], F32, kind="ExternalInput")
    w2d = nc.dram_tensor("w2d", [HID, 1], F32R, kind="ExternalInput")
    b2d = nc.dram_tensor("b2d", [1, 1], F32, kind="ExternalInput")
    dec = nc.dram_tensor("dec", [BPC, N], F32, kind="ExternalOutput")
    prob = nc.dram_tensor("prob", [BPC, N], F32, kind="ExternalOutput")

    AF = mybir.ActivationFunctionType
    ALU = mybir.AluOpType

    with tile.TileContext(nc) as tc:
        with (
            tc.tile_pool(name="consts", bufs=1) as consts,
            tc.tile_pool(name="io8", bufs=1) as io8,
            tc.tile_pool(name="sload", bufs=6) as sload,
            tc.tile_pool(name="hpool", bufs=3) as hpool,
            tc.tile_pool(name="ckpool", bufs=2) as ckpool,
            tc.tile_pool(name="phpool", bufs=2, space=bass.MemorySpace.PSUM) as phpool,
            tc.tile_pool(name="pzpool", bufs=2, space=bass.MemorySpace.PSUM) as pzpool,
        ):
            w1a = consts.tile([128, HID], F32R)
            nc.sync.dma_start(w1a[:], w1[0:128, :])
            w1b = consts.tile([128, HID], F32R)
            nc.sync.dma_start(w1b[:], w1[128:256, :])
            b1s = consts.tile([HID, 1], F32)
            nc.sync.dma_start(b1s[:], b1[:])
            w2s = consts.tile([HID, 1], F32R)
            nc.sync.dma_start(w2s[:], w2d[:])
            b2s = consts.tile([1, 1], F32)
            nc.sync.dma_start(b2s[:], b2d[:])
            rns = io8.tile([BPC, N], F32)
            nc.sync.dma_start(rns[:], rn[:])

            # per-row max of rnoise, computed up front (overlaps main loop)
            rmaxr = io8.tile([BPC, 1], F32)
            nc.vector.tensor_reduce(rmaxr[:], rns[:], mybir.AxisListType.X, ALU.max)

            # prob accumulates in [1, CHUNK] chunks on partition 0, then is
            # flushed SBUF->SBUF into row-layout pc8 (engines can only address
            # base partitions 0/32/64/96, so ACT can't write pc8 rows directly)
            pc8 = io8.tile([BPC, N], F32)

            # one-tile-deep software pipeline state
            prev = None  # (h tile, token offset) awaiting layer-2 + sigmoid
            chunk = None

            def stage2(prev, chunk):
                h, toff = prev
                pz = pzpool.tile([1, TS], F32)
                nc.tensor.matmul(pz[0:1, 0:512], w2s[:], h[:, 0:512],
                                 start=True, stop=True)
                nc.tensor.matmul(pz[0:1, 512:1024], w2s[:], h[:, 512:1024],
                                 start=True, stop=True)
                coff = toff % CHUNK
                if coff == 0:
                    chunk = ckpool.tile([1, CHUNK], F32)
                nc.scalar.activation(chunk[0:1, coff : coff + TS], pz[0:1, :],
                                     AF.Sigmoid, bias=b2s[:])
                if coff + TS == CHUNK:
                    c = toff // CHUNK
                    nc.sync.dma_start(pc8[c : c + 1, :], chunk[:])
                return chunk

            for si in range(TOK // SLAB):
                off = si * SLAB
                sa = sload.tile([128, SLAB], F32R, tag="sa")
                sb = sload.tile([128, SLAB], F32R, tag="sb")
                nc.sync.dma_start(sa[:], sT[0:128, off : off + SLAB])
                nc.sync.dma_start(sb[:], sT[128:256, off : off + SLAB])
                for half in range(SLAB // TS):
                    toff = off + half * TS
                    hoff = half * TS
                    # layer-2 of the previous tile first: its deps are older,
                    # so the in-order PE/ACT sequencers never stall on a
                    # same-tile round-trip
                    if prev is not None:
                        chunk = stage2(prev, chunk)
                    ph = phpool.tile([128, TS], F32)
                    nc.tensor.matmul(ph[:, 0:512], w1a[:],
                                     sa[:, hoff : hoff + 512],
                                     start=True, stop=False)
                    nc.tensor.matmul(ph[:, 512:1024], w1a[:],
                                     sa[:, hoff + 512 : hoff + 1024],
                                     start=True, stop=False)
                    nc.tensor.matmul(ph[:, 0:512], w1b[:],
                                     sb[:, hoff : hoff + 512],
                                     start=False, stop=True)
                    nc.tensor.matmul(ph[:, 512:1024], w1b[:],
                                     sb[:, hoff + 512 : hoff + 1024],
                                     start=False, stop=True)
                    h = hpool.tile([128, TS], F32R)
                    nc.scalar.activation(h[:], ph[:], AF.Relu, bias=b1s[:])
                    prev = (h, toff)
            chunk = stage2(prev, chunk)

            nc.sync.dma_start(prob[:], pc8[:])

            # dec = (prob > 0.5) == (z > 0); rows with no active slot get
            # argmax(rnoise) forced on. pc8/rns are updated in place to keep
            # SBUF pool footprint down (pools reserve per-partition uniformly).
            nc.vector.tensor_scalar(pc8[:], pc8[:], 0.5, None, ALU.is_gt)
            rmaxd = io8.tile([BPC, 1], F32)
            nc.vector.tensor_reduce(rmaxd[:], pc8[:], mybir.AxisListType.X, ALU.max)
            need = io8.tile([BPC, 1], F32)
            nc.vector.tensor_scalar(need[:], rmaxd[:], 0.0, None, ALU.is_equal)
            nc.vector.tensor_scalar(rns[:], rns[:], rmaxr[:], need[:],
                                    ALU.is_equal, ALU.mult)
            nc.vector.tensor_max(pc8[:], pc8[:], rns[:])
            nc.sync.dma_start(dec[:], pc8[:])

    nc.compile()
    return nc


def _get_nc():
    global _NC
    if _NC is None:
        _NC = _build_nc()
    return _NC


def _make_in_maps(s, W1, b1, W2, b2, rnoise):
    s = np.ascontiguousarray(s, dtype=np.float32)
    w1 = np.ascontiguousarray(W1, dtype=np.float32)
    b1c = np.ascontiguousarray(b1, dtype=np.float32).reshape(HID, 1)
    w2dc = np.ascontiguousarray(W2[:, 1] - W2[:, 0], dtype=np.float32).reshape(HID, 1)
    b2dc = np.array([[b2[1] - b2[0]]], dtype=np.float32)
    rn = np.ascontiguousarray(rnoise, dtype=np.float32)

    # [NCORES, D, TOK] with the contraction dim outer -> coalesced loads
    sT = np.ascontiguousarray(
        s.reshape(NCORES, TOK, D).transpose(0, 2, 1)
    )
    return [
        {
            "sT": sT[c],
            "rn": rn.reshape(NCORES, BPC, N)[c],
            "w1": w1,
            "b1": b1c,
            "w2d": w2dc,
            "b2d": b2dc,
        }
        for c in range(NCORES)
    ]


def run(s, W1, b1, W2, b2, rnoise, trace=False):
    nc = _get_nc()
    in_maps = _make_in_maps(s, W1, b1, W2, b2, rnoise)
    res = run_bass_kernel_spmd(nc, in_maps, list(range(NCORES)), trace=trace)
    dec = np.concatenate([r["dec"] for r in res.results], axis=0)
    prob = np.concatenate([r["prob"] for r in res.results], axis=0)
    return (dec, prob), res


def kernel(s, W1, b1, W2, b2, rnoise):
    (dec, prob), _ = run(s, W1, b1, W2, b2, rnoise)
    return dec, prob
